# revision 27
# baseline (speedup 1.0000x reference)
"""DigitCaps dynamic-routing kernel for 8 Trainium2 NeuronCores.

Sharding: J (num_capsule=32) split 8 ways -> 4 capsules per core, batch
replicated. Two resident W layouts: wn [(iw),(ich,q,j,p)] feeds the
S-pass and v-pass GEMMs (contraction over i); wt [(j,p),(g,q,iw)] feeds
the b-pass GEMMs (contraction over (j,p), output oriented [iw, (j,b)]
so the x-multiply uses the xq layout directly and no transpose is
needed). The routing softmax over J uses a cross-core AllReduce of
per-(b,i) partial sums of F (= exp(b) up to a shared normalizer).

Per core (j = 4 local capsules, B=64, I=2048, Q=16, P=32):
  v1 = squash(S/32),  S = sum_{i,q} W x
  t_q[iw, (j,b)] = sum_{(j',p)} wt[(j',p),(q,iw)] vbd4[(j',p),(j,b)]
  Delta[iw,(j,b)] = sum_q t_q * xq ;  F *= exp(Delta)
  Z = AllReduce_j(sum_j F);  c = F/Z
  v_k = squash(sum_{i,q} (c x) W)   [block-diag-Y GEMM, N=256]
  out = v3
"""

import numpy as np
import ml_dtypes

import concourse.bacc as bacc
import concourse.mybir as mybir
import concourse.tile as tile
from concourse.bass_utils import run_bass_kernel_spmd
from concourse.masks import make_identity

BF16 = mybir.dt.bfloat16
F32 = mybir.dt.float32
NP_BF16 = ml_dtypes.bfloat16

N_CORES = 8
B = 64
I = 2048
Q = 16
J = 32
P = 32
JL = J // N_CORES
ICH = I // 128
EPS = 1e-7
AF = mybir.ActivationFunctionType

# Transpose-mode matmul does NOT compute general MACs (verified on HW:
# garbage output) so b-pass matmuls emit fp32 PSUM; ScalarE evacuates
# to bf16 and the DVE multiplies at 2x.
TMODE = False

_CACHED = {}


def _squash(nc, small, v_sb, eps_ap):
    """In-place squash over p of v_sb [64, JL*P] fp32 (free = (j, p)).

    rsqrt is exp(-0.5*ln(s2+eps)) so every ScalarE func this kernel uses
    (Copy/Exp/Ln) lives in one activation table -- no table reloads.
    """
    sq = small.tile([B, JL * P], F32, tag="sq")
    nc.vector.tensor_mul(sq[:], v_sb[:], v_sb[:])
    red = sq.rearrange("b (j p) -> b j p", j=JL)
    w = P
    while w > 1:
        h = w // 2
        nc.vector.tensor_add(red[:, :, 0:h], red[:, :, 0:h], red[:, :, h:w])
        w = h
    s2 = small.tile([B, JL], F32, tag="s2")
    nc.vector.tensor_copy(s2[:], red[:, :, 0])
    lg = small.tile([B, JL], F32, tag="lg")
    nc.scalar.activation(lg[:], s2[:], AF.Ln, bias=eps_ap[:B, :])
    rt = small.tile([B, JL], F32, tag="rt")       # 1/sqrt(s2+eps)
    nc.scalar.activation(rt[:], lg[:], AF.Exp, scale=-0.5)
    den = small.tile([B, JL], F32, tag="den")
    nc.vector.tensor_scalar_add(den[:], s2[:], 1.0)
    rec = small.tile([B, JL], F32, tag="rec")
    nc.vector.reciprocal(rec[:], den[:])
    scale = small.tile([B, JL], F32, tag="scale")
    nc.vector.tensor_mul(scale[:], s2[:], rec[:])
    nc.vector.tensor_mul(scale[:], scale[:], rt[:])
    vv = v_sb.rearrange("b (j p) -> b j p", j=JL)
    sc_b = scale.unsqueeze(2).broadcast_to([B, JL, P])
    nc.vector.tensor_mul(vv[:], vv[:], sc_b[:])


def _build_vbd4(nc, small, psum_t, v_sb, identity):
    """v_sb [64, (j,p)] fp32 -> block-diag bf16 rhs [128, (j 4, b 64)]."""
    vt_ps = psum_t.tile([128, B], F32, tag="tp", name=f"vt_{v_sb.name}")
    nc.tensor.transpose(vt_ps[:], v_sb[:], identity[:B, :B])
    vt = small.tile([128, B], F32, tag="vt")
    nc.scalar.copy(vt[:], vt_ps[:])  # [(j,p), b]
    vbd = small.tile([128, JL * B], BF16, tag="vbd")
    nc.scalar.memzero(vbd[:])
    for j in range(JL):
        nc.scalar.copy(
            vbd[j * P:(j + 1) * P, j * B:(j + 1) * B],
            vt[j * P:(j + 1) * P, :],
        )
    return vbd


def build_kernel():
    if "nc" in _CACHED:
        return _CACHED["nc"]
    nc = bacc.Bacc(
        "TRN2", target_bir_lowering=False, debug=False, num_devices=N_CORES
    )
    wn_d = nc.dram_tensor("wn", [128, ICH * Q * JL * P], BF16, kind="ExternalInput")
    wt_d = nc.dram_tensor("wt", [128, I * Q], BF16, kind="ExternalInput")
    xq_d = nc.dram_tensor("xq", [128, ICH * Q * B], BF16, kind="ExternalInput")
    out_d = nc.dram_tensor("o", [B, JL * P], F32, kind="ExternalOutput")

    T_DT = BF16 if TMODE else F32
    TB = 1024  # t-oct free cols: 4 q x 256

    with tile.TileContext(nc) as tc:
        with (
            tc.tile_pool(name="big", bufs=1) as big,
            tc.tile_pool(name="ustr", bufs=2) as ustr,
            tc.tile_pool(name="wts", bufs=3) as wts,
            tc.tile_pool(name="small", bufs=1) as small,
            tc.tile_pool(name="ytile", bufs=2) as ytile,
            tc.tile_pool(name="dram", bufs=4, space="DRAM") as dram,
            tc.tile_pool(name="ps_v", bufs=1, space="PSUM") as ps_v,
            tc.tile_pool(name="ps_t", bufs=1, space="PSUM") as ps_t,
        ):
            # ---- resident loads, chunked so the S-pass starts early --
            xq = big.tile([128, ICH * Q * B], BF16, tag="xq")        # 32K/part
            xqv = xq.rearrange("k (ich q b) -> k ich q b", ich=ICH, q=Q)
            wn = big.tile([128, ICH * Q * JL * P], BF16, tag="wn")   # 64K/part
            CH = Q * JL * P
            XCH = Q * B
            for g in range(ICH):
                nc.sync.dma_start(
                    xq[:, g * XCH:(g + 1) * XCH], xq_d[:, g * XCH:(g + 1) * XCH]
                )
                nc.sync.dma_start(
                    wn[:, g * CH:(g + 1) * CH], wn_d[:, g * CH:(g + 1) * CH]
                )
            wnv = wn.rearrange("k (ich q j p) -> k ich q j p", ich=ICH, q=Q, j=JL)
            WCH = Q * 128

            identity = big.tile([128, 128], F32, tag="ident")
            make_identity(nc, identity[:])
            eps_t = big.tile([128, 1], F32, tag="eps")
            nc.vector.memset(eps_t[:], EPS)

            # F[iw, (ich, j, b)] bf16: running c (up to global normalizer)
            f_sb = big.tile([128, ICH * JL * B], BF16, tag="f")      # 8K/part
            f_v = f_sb.rearrange("k (ich j b) -> k ich j b", ich=ICH, j=JL)

            # warmup collective to absorb core-start skew
            wu_s = small.tile([128, 8], F32, tag="wu")
            nc.gpsimd.memset(wu_s[:], 0.0)
            wu_i = dram.tile([128, 8], F32, tag="wu_i")
            wu_o = dram.tile([128, 8], F32, tag="wu_o")
            nc.gpsimd.dma_start(wu_i[:], wu_s[:])
            nc.gpsimd.collective_compute(
                "AllReduce", mybir.AluOpType.add,
                replica_groups=[list(range(N_CORES))],
                ins=[wu_i.opt()], outs=[wu_o.opt()],
            )

            # ---- S-pass: sT[(j,p), b] = sum_{i,q} W x ---------------
            with tc.tile_pool(name="ps_s", bufs=1, space="PSUM") as ps_s:
                s_ps = ps_s.tile([128, B], F32, tag="s_ps")
                n_mm = ICH * Q
                k = 0
                for ich in range(ICH):
                    for q in range(Q):
                        nc.tensor.matmul(
                            s_ps[:],
                            wnv[:, ich, q, :, :],       # lhsT [128, (j p)]
                            xqv[:, ich, q, :],          # rhs  [128, 64]
                            start=(k == 0), stop=(k == n_mm - 1),
                        )
                        k += 1
                sT = small.tile([128, B], F32, tag="sT")
                nc.scalar.mul(sT[:], s_ps[:], 1.0 / J)
                v_ps = ps_t.tile([B, 128], F32, tag="tp", name="v_ps0")
                nc.tensor.transpose(v_ps[:], sT[:], identity[:])
                v_sb = small.tile([B, JL * P], F32, tag="v0")
                nc.scalar.copy(v_sb[:], v_ps[:])
                _squash(nc, small, v_sb, eps_t)
                vbd = _build_vbd4(nc, small, ps_t, v_sb, identity)

            # ---- 2 routing iterations -------------------------------
            NQT = 4          # collective quarters per pass
            QG = ICH // NQT  # g's per quarter
            V_LAG = 6        # b-pass g at which interleaved v-work starts
            for it in range(2):
                first = it == 0
                cc_out = [None] * NQT
                # b-pass: t_q[iw, (j,b)=256] fp32 PSUM <- 1 MM per q
                nq_oct = TB // 256
                n_oct = Q // nq_oct
                vT4 = ps_v.tile([128, JL * B], F32, tag="vT4", name=f"vT4_{it}")

                def emit_v(ich):
                    # v-pass work for one ich: quarter-head normalization,
                    # y = c*x, then 16 accumulating matmuls into vT4
                    if ich % QG == 0:
                        qt = ich // QG
                        sl = slice(qt * QG, (qt + 1) * QG)
                        zh = small.tile(
                            [128, QG * B], BF16, tag=f"z{qt}",
                            name=f"z{it}_{qt}",
                        )
                        nc.sync.dma_start(zh[:], cc_out[qt][:])
                        zr = small.tile(
                            [128, QG * B], BF16, tag=f"zr{qt}",
                            name=f"zr{it}_{qt}",
                        )
                        with nc.allow_low_precision(reason="softmax denom"):
                            nc.vector.reciprocal(zr[:], zh[:])
                        zrv = (
                            zr.rearrange("k (ic b) -> k ic b", ic=QG)
                            .unsqueeze(2).broadcast_to([128, QG, JL, B])
                        )
                        fh = f_v[:, sl, :, :]
                        nc.vector.tensor_mul(fh[:], fh[:], zrv[:])
                    y = ytile.tile(
                        [128, JL * Q * B], BF16, tag="y", bufs=2,
                        name=f"y{it}_{ich}",
                    )
                    yv = y.rearrange("k (j q b) -> k j q b", j=JL, q=Q)
                    xb = (
                        xqv[:, ich, :, :]
                        .unsqueeze(1).broadcast_to([128, JL, Q, B])
                    )
                    fb = (
                        f_v[:, ich, :, :]
                        .unsqueeze(2).broadcast_to([128, JL, Q, B])
                    )
                    nc.vector.tensor_mul(yv[:], xb[:], fb[:])
                    for q in range(Q):
                        nc.tensor.matmul(
                            vT4[:],
                            wnv[:, ich, q, :, :],     # lhsT [128,128]
                            yv[:, :, q, :],           # rhs [128,(j,b)]
                            start=(ich == 0 and q == 0),
                            stop=(ich == ICH - 1 and q == Q - 1),
                        )

                with tc.tile_pool(name=f"ps_b{it}", bufs=6 if TMODE else 3,
                                  space="PSUM") as ps_b:
                    # post-tree work is deferred one g so the ScalarE exp
                    # never head-of-line-blocks the next g's evacuations
                    def post_tree(g, ut):
                        dst = f_sb[:, g * JL * B:(g + 1) * JL * B]
                        if first:
                            nc.scalar.activation(dst, ut[:, 0, :], AF.Exp)
                        else:
                            ex = ustr.tile(
                                [128, 256], BF16, tag="ex", bufs=2,
                                name=f"e{it}_{g}",
                            )
                            nc.scalar.activation(ex[:], ut[:, 0, :], AF.Exp)
                            nc.vector.tensor_mul(dst, dst, ex[:])
                        if (g + 1) % QG == 0:
                            qt = g // QG
                            sl = slice(qt * QG, (qt + 1) * QG)
                            zph = small.tile(
                                [128, QG * B], BF16, tag=f"zp{qt}",
                                name=f"zp{it}_{qt}",
                            )
                            zpv = zph.rearrange("k (ic b) -> k ic b", ic=QG)
                            nc.vector.tensor_add(
                                zpv[:], f_v[:, sl, 0, :], f_v[:, sl, 1, :]
                            )
                            for j in range(2, JL):
                                nc.vector.tensor_add(
                                    zpv[:], zpv[:], f_v[:, sl, j, :]
                                )
                            cc_i = dram.tile(
                                [128, QG * B], BF16, tag=f"cc_i{qt}",
                                name=f"cci{it}_{qt}",
                            )
                            cc_o = dram.tile(
                                [128, QG * B], BF16, tag=f"cc_o{qt}",
                                name=f"cco{it}_{qt}",
                            )
                            nc.sync.dma_start(cc_i[:], zph[:])
                            nc.gpsimd.collective_compute(
                                "AllReduce", mybir.AluOpType.add,
                                replica_groups=[list(range(N_CORES))],
                                ins=[cc_i.opt()], outs=[cc_o.opt()],
                            )
                            cc_out[qt] = cc_o

                    pending = None
                    for g in range(ICH):
                        wt_s = wts.tile(
                            [128, WCH], BF16, tag="wt_s",
                            name=f"wt{it}_{g}",
                        )
                        nc.sync.dma_start(
                            wt_s[:], wt_d[:, g * WCH:(g + 1) * WCH]
                        )
                        # u_g [128, (q, j, b)] bf16; ue_g: evac staging
                        u_g = ustr.tile(
                            [128, Q * 256], BF16, tag="u", bufs=3,
                            name=f"u{it}_{g}",
                        )
                        ut = u_g.rearrange("k (q e) -> k q e", q=Q)
                        ue_g = ustr.tile(
                            [128, Q * 256], BF16, tag="ue", bufs=3,
                            name=f"ue{it}_{g}",
                        )
                        for oct_ in range(n_oct):
                            t_ps = ps_b.tile(
                                [128, TB], T_DT, tag="t_ps",
                                name=f"t{it}_{g}_{oct_}",
                            )
                            for qq in range(nq_oct):
                                q = oct_ * nq_oct + qq
                                nc.tensor.matmul(
                                    t_ps[:, qq * 256:(qq + 1) * 256],
                                    wt_s[:, q * 128:(q + 1) * 128],  # lhsT FWL
                                    vbd[:],                   # rhs [128,256]
                                    start=True, stop=True,
                                    is_transpose=TMODE,
                                )
                            # ScalarE evacuates fp32->bf16
                            nc.scalar.copy(
                                ue_g[:, oct_ * TB:(oct_ + 1) * TB], t_ps[:]
                            )
                        # one DVE mult for the whole g at 2x
                        xv = (
                            xqv[:, g, :, :]
                            .unsqueeze(2).broadcast_to([128, Q, JL, B])
                        )
                        uev = ue_g.rearrange(
                            "k (q j b) -> k q j b", q=Q, j=JL
                        )
                        uqv = u_g.rearrange(
                            "k (q j b) -> k q j b", q=Q, j=JL
                        )
                        nc.vector.tensor_mul(uqv[:], uev[:], xv[:])
                        # q-tree: 4 strided adds -> Delta = ut[:, 0, :]
                        w = Q
                        while w > 1:
                            h = w // 2
                            nc.vector.tensor_add(
                                ut[:, 0:h, :], ut[:, 0:h, :], ut[:, h:w, :],
                            )
                            w = h
                        if pending is not None:
                            post_tree(*pending)
                        pending = (g, ut)
                        # interleave v-pass work for ready quarters
                        if g >= V_LAG:
                            emit_v(g - V_LAG)
                    post_tree(*pending)
                    for ich in range(ICH - V_LAG, ICH):
                        emit_v(ich)
                # diag-block evac -> vT [128, 64] -> transpose -> squash
                vT = small.tile([128, B], F32, tag="vTd", name=f"vTd{it}")
                for j in range(JL):
                    nc.scalar.copy(
                        vT[j * P:(j + 1) * P, :],
                        vT4[j * P:(j + 1) * P, j * B:(j + 1) * B],
                    )
                v_ps = ps_t.tile([B, 128], F32, tag="tp", name=f"vps{it}")
                nc.tensor.transpose(v_ps[:], vT[:], identity[:])
                v_sb = small.tile([B, JL * P], F32, tag="v", name=f"v{it}")
                nc.scalar.copy(v_sb[:], v_ps[:])
                _squash(nc, small, v_sb, eps_t)
                if it == 0:
                    vbd = _build_vbd4(nc, small, ps_t, v_sb, identity)
                else:
                    nc.sync.dma_start(out_d[:], v_sb[:])

    nc.compile()
    _CACHED["nc"] = nc
    return nc


def _prep_inputs(inputs_np, W_np):
    x = np.ascontiguousarray(inputs_np)           # [B, I, Q] f32
    W = np.ascontiguousarray(W_np)                # [J, I, P, Q] f32
    xq = (
        x.reshape(B, ICH, 128, Q).transpose(2, 1, 3, 0)
        .astype(NP_BF16).reshape(128, ICH * Q * B)
    )
    in_maps = []
    for r in range(N_CORES):
        Wr = W[r * JL:(r + 1) * JL]                       # [4, I, P, Q]
        wn = (
            Wr.reshape(JL, ICH, 128, P, Q).transpose(2, 1, 4, 0, 3)
            .astype(NP_BF16).reshape(128, ICH * Q * JL * P)
        )
        wt = (
            Wr.reshape(JL, ICH, 128, P, Q)
            .transpose(0, 3, 1, 4, 2)                     # [j, p, g, q, iw]
            .astype(NP_BF16).reshape(128, I * Q)
        )
        in_maps.append(
            {
                "wn": np.ascontiguousarray(wn),
                "wt": np.ascontiguousarray(wt),
                "xq": np.ascontiguousarray(xq),
            }
        )
    return in_maps


def kernel(inputs, W, _trace=False):
    nc = build_kernel()
    in_maps = _prep_inputs(np.asarray(inputs), np.asarray(W))
    res = run_bass_kernel_spmd(nc, in_maps, list(range(N_CORES)), trace=_trace)
    out = np.concatenate(
        [res.results[r]["o"].reshape(B, JL, P) for r in range(N_CORES)], axis=1
    )
    if _trace:
        kernel.last_exec_ns = res.exec_time_ns
        kernel.last_results = res
    return out.astype(np.float32)


# revision 28
# speedup vs baseline: 1.0362x; 1.0362x over previous
"""DigitCaps dynamic-routing kernel for 8 Trainium2 NeuronCores.

Sharding: J (num_capsule=32) split 8 ways -> 4 capsules per core, batch
replicated. Two resident W layouts: wn [(iw),(ich,q,j,p)] feeds the
S-pass and v-pass GEMMs (contraction over i); wt [(j,p),(g,q,iw)] feeds
the b-pass GEMMs (contraction over (j,p), output oriented [iw, (j,b)]
so the x-multiply uses the xq layout directly and no transpose is
needed). The routing softmax over J uses a cross-core AllReduce of
per-(b,i) partial sums of F (= exp(b) up to a shared normalizer).

Per core (j = 4 local capsules, B=64, I=2048, Q=16, P=32):
  v1 = squash(S/32),  S = sum_{i,q} W x
  t_q[iw, (j,b)] = sum_{(j',p)} wt[(j',p),(q,iw)] vbd4[(j',p),(j,b)]
  Delta[iw,(j,b)] = sum_q t_q * xq ;  F *= exp(Delta)
  Z = AllReduce_j(sum_j F);  c = F/Z
  v_k = squash(sum_{i,q} (c x) W)   [block-diag-Y GEMM, N=256]
  out = v3
"""

import numpy as np
import ml_dtypes

import concourse.bacc as bacc
import concourse.mybir as mybir
import concourse.tile as tile
from concourse.bass_utils import run_bass_kernel_spmd
from concourse.masks import make_identity

BF16 = mybir.dt.bfloat16
F32 = mybir.dt.float32
NP_BF16 = ml_dtypes.bfloat16

N_CORES = 8
B = 64
I = 2048
Q = 16
J = 32
P = 32
JL = J // N_CORES
ICH = I // 128
EPS = 1e-7
AF = mybir.ActivationFunctionType

# Transpose-mode matmul does NOT compute general MACs (verified on HW:
# garbage output) so b-pass matmuls emit fp32 PSUM; ScalarE evacuates
# to bf16 and the DVE multiplies at 2x.
TMODE = False

_CACHED = {}


def _squash(nc, small, v_sb, eps_ap):
    """In-place squash over p of v_sb [64, JL*P] fp32 (free = (j, p)).

    rsqrt is exp(-0.5*ln(s2+eps)) so every ScalarE func this kernel uses
    (Copy/Exp/Ln) lives in one activation table -- no table reloads.
    """
    sq = small.tile([B, JL * P], F32, tag="sq")
    nc.vector.tensor_mul(sq[:], v_sb[:], v_sb[:])
    red = sq.rearrange("b (j p) -> b j p", j=JL)
    w = P
    while w > 1:
        h = w // 2
        nc.vector.tensor_add(red[:, :, 0:h], red[:, :, 0:h], red[:, :, h:w])
        w = h
    s2 = small.tile([B, JL], F32, tag="s2")
    nc.vector.tensor_copy(s2[:], red[:, :, 0])
    lg = small.tile([B, JL], F32, tag="lg")
    nc.scalar.activation(lg[:], s2[:], AF.Ln, bias=eps_ap[:B, :])
    rt = small.tile([B, JL], F32, tag="rt")       # 1/sqrt(s2+eps)
    nc.scalar.activation(rt[:], lg[:], AF.Exp, scale=-0.5)
    den = small.tile([B, JL], F32, tag="den")
    nc.vector.tensor_scalar_add(den[:], s2[:], 1.0)
    rec = small.tile([B, JL], F32, tag="rec")
    nc.vector.reciprocal(rec[:], den[:])
    scale = small.tile([B, JL], F32, tag="scale")
    nc.vector.tensor_mul(scale[:], s2[:], rec[:])
    nc.vector.tensor_mul(scale[:], scale[:], rt[:])
    vv = v_sb.rearrange("b (j p) -> b j p", j=JL)
    sc_b = scale.unsqueeze(2).broadcast_to([B, JL, P])
    nc.vector.tensor_mul(vv[:], vv[:], sc_b[:])


def _build_vbd4(nc, small, psum_t, v_sb, identity):
    """v_sb [64, (j,p)] fp32 -> block-diag bf16 rhs [128, (j 4, b 64)]."""
    vt_ps = psum_t.tile([128, B], F32, tag="tp", name=f"vt_{v_sb.name}")
    nc.tensor.transpose(vt_ps[:], v_sb[:], identity[:B, :B])
    vt = small.tile([128, B], F32, tag="vt")
    nc.scalar.copy(vt[:], vt_ps[:])  # [(j,p), b]
    vbd = small.tile([128, JL * B], BF16, tag="vbd")
    nc.scalar.memzero(vbd[:])
    for j in range(JL):
        nc.scalar.copy(
            vbd[j * P:(j + 1) * P, j * B:(j + 1) * B],
            vt[j * P:(j + 1) * P, :],
        )
    return vbd


def build_kernel():
    if "nc" in _CACHED:
        return _CACHED["nc"]
    nc = bacc.Bacc(
        "TRN2", target_bir_lowering=False, debug=False, num_devices=N_CORES
    )
    wn_d = nc.dram_tensor("wn", [128, ICH * Q * JL * P], BF16, kind="ExternalInput")
    wt_d = nc.dram_tensor("wt", [128, I * Q], BF16, kind="ExternalInput")
    xq_d = nc.dram_tensor("xq", [128, ICH * Q * B], BF16, kind="ExternalInput")
    out_d = nc.dram_tensor("o", [B, JL * P], F32, kind="ExternalOutput")

    T_DT = BF16 if TMODE else F32
    TB = 1024  # t-oct free cols: 4 q x 256

    with tile.TileContext(nc) as tc:
        with (
            tc.tile_pool(name="big", bufs=1) as big,
            tc.tile_pool(name="ustr", bufs=2) as ustr,
            tc.tile_pool(name="wts", bufs=3) as wts,
            tc.tile_pool(name="small", bufs=1) as small,
            tc.tile_pool(name="ytile", bufs=2) as ytile,
            tc.tile_pool(name="dram", bufs=4, space="DRAM") as dram,
            tc.tile_pool(name="ps_v", bufs=1, space="PSUM") as ps_v,
            tc.tile_pool(name="ps_t", bufs=1, space="PSUM") as ps_t,
        ):
            # ---- resident loads, chunked so the S-pass starts early --
            xq = big.tile([128, ICH * Q * B], BF16, tag="xq")        # 32K/part
            xqv = xq.rearrange("k (ich q b) -> k ich q b", ich=ICH, q=Q)
            wn = big.tile([128, ICH * Q * JL * P], BF16, tag="wn")   # 64K/part
            CH = Q * JL * P
            XCH = Q * B
            for g in range(ICH):
                nc.sync.dma_start(
                    xq[:, g * XCH:(g + 1) * XCH], xq_d[:, g * XCH:(g + 1) * XCH]
                )
                nc.sync.dma_start(
                    wn[:, g * CH:(g + 1) * CH], wn_d[:, g * CH:(g + 1) * CH]
                )
            wnv = wn.rearrange("k (ich q j p) -> k ich q j p", ich=ICH, q=Q, j=JL)
            WCH = Q * 128

            identity = big.tile([128, 128], F32, tag="ident")
            make_identity(nc, identity[:])
            eps_t = big.tile([128, 1], F32, tag="eps")
            nc.vector.memset(eps_t[:], EPS)

            # F[iw, (ich, j, b)] bf16: running c (up to global normalizer)
            f_sb = big.tile([128, ICH * JL * B], BF16, tag="f")      # 8K/part
            f_v = f_sb.rearrange("k (ich j b) -> k ich j b", ich=ICH, j=JL)

            # warmup collective to absorb core-start skew
            wu_s = small.tile([128, 8], F32, tag="wu")
            nc.gpsimd.memset(wu_s[:], 0.0)
            wu_i = dram.tile([128, 8], F32, tag="wu_i")
            wu_o = dram.tile([128, 8], F32, tag="wu_o")
            nc.gpsimd.dma_start(wu_i[:], wu_s[:])
            nc.gpsimd.collective_compute(
                "AllReduce", mybir.AluOpType.add,
                replica_groups=[list(range(N_CORES))],
                ins=[wu_i.opt()], outs=[wu_o.opt()],
            )

            # ---- S-pass: sT[(j,p), b] = sum_{i,q} W x ---------------
            with tc.tile_pool(name="ps_s", bufs=1, space="PSUM") as ps_s:
                s_ps = ps_s.tile([128, B], F32, tag="s_ps")
                n_mm = ICH * Q
                k = 0
                for ich in range(ICH):
                    for q in range(Q):
                        nc.tensor.matmul(
                            s_ps[:],
                            wnv[:, ich, q, :, :],       # lhsT [128, (j p)]
                            xqv[:, ich, q, :],          # rhs  [128, 64]
                            start=(k == 0), stop=(k == n_mm - 1),
                        )
                        k += 1
                sT = small.tile([128, B], F32, tag="sT")
                nc.scalar.mul(sT[:], s_ps[:], 1.0 / J)
                v_ps = ps_t.tile([B, 128], F32, tag="tp", name="v_ps0")
                nc.tensor.transpose(v_ps[:], sT[:], identity[:])
                v_sb = small.tile([B, JL * P], F32, tag="v0")
                nc.scalar.copy(v_sb[:], v_ps[:])
                _squash(nc, small, v_sb, eps_t)
                vbd = _build_vbd4(nc, small, ps_t, v_sb, identity)

            # ---- 2 routing iterations -------------------------------
            NQT = 4          # collective quarters per pass
            QG = ICH // NQT  # g's per quarter
            V_LAG = 8        # b-pass g at which interleaved v-work starts
            for it in range(2):
                first = it == 0
                cc_out = [None] * NQT
                # b-pass: t_q[iw, (j,b)=256] fp32 PSUM <- 1 MM per q
                nq_oct = TB // 256
                n_oct = Q // nq_oct
                vT4 = ps_v.tile([128, JL * B], F32, tag="vT4", name=f"vT4_{it}")

                def emit_v(ich):
                    # v-pass work for one ich: quarter-head normalization,
                    # y = c*x, then 16 accumulating matmuls into vT4
                    if ich % QG == 0:
                        qt = ich // QG
                        sl = slice(qt * QG, (qt + 1) * QG)
                        zh = small.tile(
                            [128, QG * B], BF16, tag=f"z{qt}",
                            name=f"z{it}_{qt}",
                        )
                        nc.sync.dma_start(zh[:], cc_out[qt][:])
                        zr = small.tile(
                            [128, QG * B], BF16, tag=f"zr{qt}",
                            name=f"zr{it}_{qt}",
                        )
                        with nc.allow_low_precision(reason="softmax denom"):
                            nc.vector.reciprocal(zr[:], zh[:])
                        zrv = (
                            zr.rearrange("k (ic b) -> k ic b", ic=QG)
                            .unsqueeze(2).broadcast_to([128, QG, JL, B])
                        )
                        fh = f_v[:, sl, :, :]
                        nc.vector.tensor_mul(fh[:], fh[:], zrv[:])
                    y = ytile.tile(
                        [128, JL * Q * B], BF16, tag="y", bufs=2,
                        name=f"y{it}_{ich}",
                    )
                    yv = y.rearrange("k (j q b) -> k j q b", j=JL, q=Q)
                    xb = (
                        xqv[:, ich, :, :]
                        .unsqueeze(1).broadcast_to([128, JL, Q, B])
                    )
                    fb = (
                        f_v[:, ich, :, :]
                        .unsqueeze(2).broadcast_to([128, JL, Q, B])
                    )
                    nc.vector.tensor_mul(yv[:], xb[:], fb[:])
                    for q in range(Q):
                        nc.tensor.matmul(
                            vT4[:],
                            wnv[:, ich, q, :, :],     # lhsT [128,128]
                            yv[:, :, q, :],           # rhs [128,(j,b)]
                            start=(ich == 0 and q == 0),
                            stop=(ich == ICH - 1 and q == Q - 1),
                        )

                with tc.tile_pool(name=f"ps_b{it}", bufs=6 if TMODE else 3,
                                  space="PSUM") as ps_b:
                    # post-tree work is deferred one g so the ScalarE exp
                    # never head-of-line-blocks the next g's evacuations
                    def post_tree(g, ut):
                        dst = f_sb[:, g * JL * B:(g + 1) * JL * B]
                        if first:
                            nc.scalar.activation(dst, ut[:, 0, :], AF.Exp)
                        else:
                            ex = ustr.tile(
                                [128, 256], BF16, tag="ex", bufs=2,
                                name=f"e{it}_{g}",
                            )
                            nc.scalar.activation(ex[:], ut[:, 0, :], AF.Exp)
                            nc.vector.tensor_mul(dst, dst, ex[:])
                        if (g + 1) % QG == 0:
                            qt = g // QG
                            sl = slice(qt * QG, (qt + 1) * QG)
                            zph = small.tile(
                                [128, QG * B], BF16, tag=f"zp{qt}",
                                name=f"zp{it}_{qt}",
                            )
                            zpv = zph.rearrange("k (ic b) -> k ic b", ic=QG)
                            nc.vector.tensor_add(
                                zpv[:], f_v[:, sl, 0, :], f_v[:, sl, 1, :]
                            )
                            for j in range(2, JL):
                                nc.vector.tensor_add(
                                    zpv[:], zpv[:], f_v[:, sl, j, :]
                                )
                            cc_i = dram.tile(
                                [128, QG * B], BF16, tag=f"cc_i{qt}",
                                name=f"cci{it}_{qt}",
                            )
                            cc_o = dram.tile(
                                [128, QG * B], BF16, tag=f"cc_o{qt}",
                                name=f"cco{it}_{qt}",
                            )
                            nc.sync.dma_start(cc_i[:], zph[:])
                            nc.gpsimd.collective_compute(
                                "AllReduce", mybir.AluOpType.add,
                                replica_groups=[list(range(N_CORES))],
                                ins=[cc_i.opt()], outs=[cc_o.opt()],
                            )
                            cc_out[qt] = cc_o

                    pending = None
                    for g in range(ICH):
                        wt_s = wts.tile(
                            [128, WCH], BF16, tag="wt_s",
                            name=f"wt{it}_{g}",
                        )
                        nc.sync.dma_start(
                            wt_s[:], wt_d[:, g * WCH:(g + 1) * WCH]
                        )
                        # u_g [128, (q, j, b)] bf16; ue_g: evac staging
                        u_g = ustr.tile(
                            [128, Q * 256], BF16, tag="u", bufs=3,
                            name=f"u{it}_{g}",
                        )
                        ut = u_g.rearrange("k (q e) -> k q e", q=Q)
                        ue_g = ustr.tile(
                            [128, Q * 256], BF16, tag="ue", bufs=3,
                            name=f"ue{it}_{g}",
                        )
                        for oct_ in range(n_oct):
                            t_ps = ps_b.tile(
                                [128, TB], T_DT, tag="t_ps",
                                name=f"t{it}_{g}_{oct_}",
                            )
                            for qq in range(nq_oct):
                                q = oct_ * nq_oct + qq
                                nc.tensor.matmul(
                                    t_ps[:, qq * 256:(qq + 1) * 256],
                                    wt_s[:, q * 128:(q + 1) * 128],  # lhsT FWL
                                    vbd[:],                   # rhs [128,256]
                                    start=True, stop=True,
                                    is_transpose=TMODE,
                                )
                            # ScalarE evacuates fp32->bf16
                            nc.scalar.copy(
                                ue_g[:, oct_ * TB:(oct_ + 1) * TB], t_ps[:]
                            )
                        # one DVE mult for the whole g at 2x
                        xv = (
                            xqv[:, g, :, :]
                            .unsqueeze(2).broadcast_to([128, Q, JL, B])
                        )
                        uev = ue_g.rearrange(
                            "k (q j b) -> k q j b", q=Q, j=JL
                        )
                        uqv = u_g.rearrange(
                            "k (q j b) -> k q j b", q=Q, j=JL
                        )
                        nc.vector.tensor_mul(uqv[:], uev[:], xv[:])
                        # q-tree: 4 strided adds -> Delta = ut[:, 0, :]
                        w = Q
                        while w > 1:
                            h = w // 2
                            nc.vector.tensor_add(
                                ut[:, 0:h, :], ut[:, 0:h, :], ut[:, h:w, :],
                            )
                            w = h
                        if pending is not None:
                            post_tree(*pending)
                        pending = (g, ut)
                        # interleave v-pass work for ready quarters
                        if g >= V_LAG:
                            emit_v(g - V_LAG)
                    post_tree(*pending)
                    for ich in range(ICH - V_LAG, ICH):
                        emit_v(ich)
                # diag-block evac -> vT [128, 64] -> transpose -> squash
                vT = small.tile([128, B], F32, tag="vTd", name=f"vTd{it}")
                for j in range(JL):
                    nc.scalar.copy(
                        vT[j * P:(j + 1) * P, :],
                        vT4[j * P:(j + 1) * P, j * B:(j + 1) * B],
                    )
                v_ps = ps_t.tile([B, 128], F32, tag="tp", name=f"vps{it}")
                nc.tensor.transpose(v_ps[:], vT[:], identity[:])
                v_sb = small.tile([B, JL * P], F32, tag="v", name=f"v{it}")
                nc.scalar.copy(v_sb[:], v_ps[:])
                _squash(nc, small, v_sb, eps_t)
                if it == 0:
                    vbd = _build_vbd4(nc, small, ps_t, v_sb, identity)
                else:
                    nc.sync.dma_start(out_d[:], v_sb[:])

    nc.compile()
    _CACHED["nc"] = nc
    return nc


def _prep_inputs(inputs_np, W_np):
    x = np.ascontiguousarray(inputs_np)           # [B, I, Q] f32
    W = np.ascontiguousarray(W_np)                # [J, I, P, Q] f32
    xq = (
        x.reshape(B, ICH, 128, Q).transpose(2, 1, 3, 0)
        .astype(NP_BF16).reshape(128, ICH * Q * B)
    )
    in_maps = []
    for r in range(N_CORES):
        Wr = W[r * JL:(r + 1) * JL]                       # [4, I, P, Q]
        wn = (
            Wr.reshape(JL, ICH, 128, P, Q).transpose(2, 1, 4, 0, 3)
            .astype(NP_BF16).reshape(128, ICH * Q * JL * P)
        )
        wt = (
            Wr.reshape(JL, ICH, 128, P, Q)
            .transpose(0, 3, 1, 4, 2)                     # [j, p, g, q, iw]
            .astype(NP_BF16).reshape(128, I * Q)
        )
        in_maps.append(
            {
                "wn": np.ascontiguousarray(wn),
                "wt": np.ascontiguousarray(wt),
                "xq": np.ascontiguousarray(xq),
            }
        )
    return in_maps


def kernel(inputs, W, _trace=False):
    nc = build_kernel()
    in_maps = _prep_inputs(np.asarray(inputs), np.asarray(W))
    res = run_bass_kernel_spmd(nc, in_maps, list(range(N_CORES)), trace=_trace)
    out = np.concatenate(
        [res.results[r]["o"].reshape(B, JL, P) for r in range(N_CORES)], axis=1
    )
    if _trace:
        kernel.last_exec_ns = res.exec_time_ns
        kernel.last_results = res
    return out.astype(np.float32)


# revision 29
# speedup vs baseline: 1.0642x; 1.0270x over previous
"""DigitCaps dynamic-routing kernel for 8 Trainium2 NeuronCores.

Sharding: J (num_capsule=32) split 8 ways -> 4 capsules per core, batch
replicated. Two resident W layouts: wn [(iw),(ich,q,j,p)] feeds the
S-pass and v-pass GEMMs (contraction over i); wt [(j,p),(g,q,iw)] feeds
the b-pass GEMMs (contraction over (j,p), output oriented [iw, (j,b)]
so the x-multiply uses the xq layout directly and no transpose is
needed). The routing softmax over J uses a cross-core AllReduce of
per-(b,i) partial sums of F (= exp(b) up to a shared normalizer).

Per core (j = 4 local capsules, B=64, I=2048, Q=16, P=32):
  v1 = squash(S/32),  S = sum_{i,q} W x
  t_q[iw, (j,b)] = sum_{(j',p)} wt[(j',p),(q,iw)] vbd4[(j',p),(j,b)]
  Delta[iw,(j,b)] = sum_q t_q * xq ;  F *= exp(Delta)
  Z = AllReduce_j(sum_j F);  c = F/Z
  v_k = squash(sum_{i,q} (c x) W)   [block-diag-Y GEMM, N=256]
  out = v3
"""

import numpy as np
import ml_dtypes

import concourse.bacc as bacc
import concourse.mybir as mybir
import concourse.tile as tile
from concourse.bass_utils import run_bass_kernel_spmd
from concourse.masks import make_identity

BF16 = mybir.dt.bfloat16
F32 = mybir.dt.float32
NP_BF16 = ml_dtypes.bfloat16

N_CORES = 8
B = 64
I = 2048
Q = 16
J = 32
P = 32
JL = J // N_CORES
ICH = I // 128
EPS = 1e-7
AF = mybir.ActivationFunctionType

# Transpose-mode matmul does NOT compute general MACs (verified on HW:
# garbage output) so b-pass matmuls emit fp32 PSUM; ScalarE evacuates
# to bf16 and the DVE multiplies at 2x.
TMODE = False

_CACHED = {}


def _squash(nc, small, v_sb, eps_ap):
    """In-place squash over p of v_sb [64, JL*P] fp32 (free = (j, p)).

    rsqrt is exp(-0.5*ln(s2+eps)) so every ScalarE func this kernel uses
    (Copy/Exp/Ln) lives in one activation table -- no table reloads.
    """
    sq = small.tile([B, JL * P], F32, tag="sq")
    nc.vector.tensor_mul(sq[:], v_sb[:], v_sb[:])
    red = sq.rearrange("b (j p) -> b j p", j=JL)
    w = P
    while w > 1:
        h = w // 2
        nc.vector.tensor_add(red[:, :, 0:h], red[:, :, 0:h], red[:, :, h:w])
        w = h
    s2 = small.tile([B, JL], F32, tag="s2")
    nc.vector.tensor_copy(s2[:], red[:, :, 0])
    lg = small.tile([B, JL], F32, tag="lg")
    nc.scalar.activation(lg[:], s2[:], AF.Ln, bias=eps_ap[:B, :])
    rt = small.tile([B, JL], F32, tag="rt")       # 1/sqrt(s2+eps)
    nc.scalar.activation(rt[:], lg[:], AF.Exp, scale=-0.5)
    den = small.tile([B, JL], F32, tag="den")
    nc.vector.tensor_scalar_add(den[:], s2[:], 1.0)
    rec = small.tile([B, JL], F32, tag="rec")
    nc.vector.reciprocal(rec[:], den[:])
    scale = small.tile([B, JL], F32, tag="scale")
    nc.vector.tensor_mul(scale[:], s2[:], rec[:])
    nc.vector.tensor_mul(scale[:], scale[:], rt[:])
    vv = v_sb.rearrange("b (j p) -> b j p", j=JL)
    sc_b = scale.unsqueeze(2).broadcast_to([B, JL, P])
    nc.vector.tensor_mul(vv[:], vv[:], sc_b[:])


def _build_vbd4(nc, small, psum_t, v_sb, identity):
    """v_sb [64, (j,p)] fp32 -> block-diag bf16 rhs [128, (j 4, b 64)]."""
    vt_ps = psum_t.tile([128, B], F32, tag="tp", name=f"vt_{v_sb.name}")
    nc.tensor.transpose(vt_ps[:], v_sb[:], identity[:B, :B])
    vt = small.tile([128, B], F32, tag="vt")
    nc.scalar.copy(vt[:], vt_ps[:])  # [(j,p), b]
    vbd = small.tile([128, JL * B], BF16, tag="vbd")
    nc.scalar.memzero(vbd[:])
    for j in range(JL):
        nc.scalar.copy(
            vbd[j * P:(j + 1) * P, j * B:(j + 1) * B],
            vt[j * P:(j + 1) * P, :],
        )
    return vbd


def build_kernel():
    if "nc" in _CACHED:
        return _CACHED["nc"]
    nc = bacc.Bacc(
        "TRN2", target_bir_lowering=False, debug=False, num_devices=N_CORES
    )
    wn_d = nc.dram_tensor("wn", [128, ICH * Q * JL * P], BF16, kind="ExternalInput")
    wt_d = nc.dram_tensor("wt", [128, I * Q], BF16, kind="ExternalInput")
    xq_d = nc.dram_tensor("xq", [128, ICH * Q * B], BF16, kind="ExternalInput")
    out_d = nc.dram_tensor("o", [B, JL * P], F32, kind="ExternalOutput")

    T_DT = BF16 if TMODE else F32
    TB = 1024  # t-oct free cols: 4 q x 256

    with tile.TileContext(nc) as tc:
        with (
            tc.tile_pool(name="big", bufs=1) as big,
            tc.tile_pool(name="ustr", bufs=2) as ustr,
            tc.tile_pool(name="wts", bufs=4) as wts,
            tc.tile_pool(name="small", bufs=1) as small,
            tc.tile_pool(name="ytile", bufs=2) as ytile,
            tc.tile_pool(name="dram", bufs=4, space="DRAM") as dram,
            tc.tile_pool(name="ps_v", bufs=1, space="PSUM") as ps_v,
            tc.tile_pool(name="ps_t", bufs=1, space="PSUM") as ps_t,
        ):
            # ---- resident loads, chunked so the S-pass starts early --
            xq = big.tile([128, ICH * Q * B], BF16, tag="xq")        # 32K/part
            xqv = xq.rearrange("k (ich q b) -> k ich q b", ich=ICH, q=Q)
            wn = big.tile([128, ICH * Q * JL * P], BF16, tag="wn")   # 64K/part
            CH = Q * JL * P
            XCH = Q * B
            for g in range(ICH):
                nc.sync.dma_start(
                    xq[:, g * XCH:(g + 1) * XCH], xq_d[:, g * XCH:(g + 1) * XCH]
                )
                nc.sync.dma_start(
                    wn[:, g * CH:(g + 1) * CH], wn_d[:, g * CH:(g + 1) * CH]
                )
            wnv = wn.rearrange("k (ich q j p) -> k ich q j p", ich=ICH, q=Q, j=JL)
            WCH = Q * 128

            identity = big.tile([128, 128], F32, tag="ident")
            make_identity(nc, identity[:])
            eps_t = big.tile([128, 1], F32, tag="eps")
            nc.vector.memset(eps_t[:], EPS)

            # F[iw, (ich, j, b)] bf16: running c (up to global normalizer)
            f_sb = big.tile([128, ICH * JL * B], BF16, tag="f")      # 8K/part
            f_v = f_sb.rearrange("k (ich j b) -> k ich j b", ich=ICH, j=JL)

            # warmup collective to absorb core-start skew
            wu_s = small.tile([128, 8], F32, tag="wu")
            nc.gpsimd.memset(wu_s[:], 0.0)
            wu_i = dram.tile([128, 8], F32, tag="wu_i")
            wu_o = dram.tile([128, 8], F32, tag="wu_o")
            nc.gpsimd.dma_start(wu_i[:], wu_s[:])
            nc.gpsimd.collective_compute(
                "AllReduce", mybir.AluOpType.add,
                replica_groups=[list(range(N_CORES))],
                ins=[wu_i.opt()], outs=[wu_o.opt()],
            )

            # ---- S-pass: sT[(j,p), b] = sum_{i,q} W x ---------------
            with tc.tile_pool(name="ps_s", bufs=1, space="PSUM") as ps_s:
                s_ps = ps_s.tile([128, B], F32, tag="s_ps")
                n_mm = ICH * Q
                k = 0
                for ich in range(ICH):
                    for q in range(Q):
                        nc.tensor.matmul(
                            s_ps[:],
                            wnv[:, ich, q, :, :],       # lhsT [128, (j p)]
                            xqv[:, ich, q, :],          # rhs  [128, 64]
                            start=(k == 0), stop=(k == n_mm - 1),
                        )
                        k += 1
                sT = small.tile([128, B], F32, tag="sT")
                nc.scalar.mul(sT[:], s_ps[:], 1.0 / J)
                v_ps = ps_t.tile([B, 128], F32, tag="tp", name="v_ps0")
                nc.tensor.transpose(v_ps[:], sT[:], identity[:])
                v_sb = small.tile([B, JL * P], F32, tag="v0")
                nc.scalar.copy(v_sb[:], v_ps[:])
                _squash(nc, small, v_sb, eps_t)
                vbd = _build_vbd4(nc, small, ps_t, v_sb, identity)

            # ---- 2 routing iterations -------------------------------
            NQT = 4          # collective quarters per pass
            QG = ICH // NQT  # g's per quarter
            V_LAG = 8        # b-pass g at which interleaved v-work starts
            for it in range(2):
                first = it == 0
                cc_out = [None] * NQT
                # b-pass: t_q[iw, (j,b)=256] fp32 PSUM <- 1 MM per q
                nq_oct = TB // 256
                n_oct = Q // nq_oct
                vT4 = ps_v.tile([128, JL * B], F32, tag="vT4", name=f"vT4_{it}")

                def emit_v(ich):
                    # v-pass work for one ich: quarter-head normalization,
                    # y = c*x, then 16 accumulating matmuls into vT4
                    if ich % QG == 0:
                        qt = ich // QG
                        sl = slice(qt * QG, (qt + 1) * QG)
                        zh = small.tile(
                            [128, QG * B], BF16, tag=f"z{qt}",
                            name=f"z{it}_{qt}",
                        )
                        nc.sync.dma_start(zh[:], cc_out[qt][:])
                        zr = small.tile(
                            [128, QG * B], BF16, tag=f"zr{qt}",
                            name=f"zr{it}_{qt}",
                        )
                        with nc.allow_low_precision(reason="softmax denom"):
                            nc.vector.reciprocal(zr[:], zh[:])
                        zrv = (
                            zr.rearrange("k (ic b) -> k ic b", ic=QG)
                            .unsqueeze(2).broadcast_to([128, QG, JL, B])
                        )
                        fh = f_v[:, sl, :, :]
                        nc.vector.tensor_mul(fh[:], fh[:], zrv[:])
                    y = ytile.tile(
                        [128, JL * Q * B], BF16, tag="y", bufs=3,
                        name=f"y{it}_{ich}",
                    )
                    yv = y.rearrange("k (j q b) -> k j q b", j=JL, q=Q)
                    xb = (
                        xqv[:, ich, :, :]
                        .unsqueeze(1).broadcast_to([128, JL, Q, B])
                    )
                    fb = (
                        f_v[:, ich, :, :]
                        .unsqueeze(2).broadcast_to([128, JL, Q, B])
                    )
                    nc.vector.tensor_mul(yv[:], xb[:], fb[:])
                    for q in range(Q):
                        nc.tensor.matmul(
                            vT4[:],
                            wnv[:, ich, q, :, :],     # lhsT [128,128]
                            yv[:, :, q, :],           # rhs [128,(j,b)]
                            start=(ich == 0 and q == 0),
                            stop=(ich == ICH - 1 and q == Q - 1),
                        )

                with tc.tile_pool(name=f"ps_b{it}", bufs=6 if TMODE else 3,
                                  space="PSUM") as ps_b:
                    # post-tree work is deferred one g so the ScalarE exp
                    # never head-of-line-blocks the next g's evacuations
                    def post_tree(g, ut):
                        dst = f_sb[:, g * JL * B:(g + 1) * JL * B]
                        if first:
                            nc.scalar.activation(dst, ut[:, 0, :], AF.Exp)
                        else:
                            ex = ustr.tile(
                                [128, 256], BF16, tag="ex", bufs=2,
                                name=f"e{it}_{g}",
                            )
                            nc.scalar.activation(ex[:], ut[:, 0, :], AF.Exp)
                            nc.vector.tensor_mul(dst, dst, ex[:])
                        if (g + 1) % QG == 0:
                            qt = g // QG
                            sl = slice(qt * QG, (qt + 1) * QG)
                            zph = small.tile(
                                [128, QG * B], BF16, tag=f"zp{qt}",
                                name=f"zp{it}_{qt}",
                            )
                            zpv = zph.rearrange("k (ic b) -> k ic b", ic=QG)
                            nc.vector.tensor_add(
                                zpv[:], f_v[:, sl, 0, :], f_v[:, sl, 1, :]
                            )
                            for j in range(2, JL):
                                nc.vector.tensor_add(
                                    zpv[:], zpv[:], f_v[:, sl, j, :]
                                )
                            cc_i = dram.tile(
                                [128, QG * B], BF16, tag=f"cc_i{qt}",
                                name=f"cci{it}_{qt}",
                            )
                            cc_o = dram.tile(
                                [128, QG * B], BF16, tag=f"cc_o{qt}",
                                name=f"cco{it}_{qt}",
                            )
                            nc.sync.dma_start(cc_i[:], zph[:])
                            nc.gpsimd.collective_compute(
                                "AllReduce", mybir.AluOpType.add,
                                replica_groups=[list(range(N_CORES))],
                                ins=[cc_i.opt()], outs=[cc_o.opt()],
                            )
                            cc_out[qt] = cc_o

                    pending = None
                    for g in range(ICH):
                        wt_s = wts.tile(
                            [128, WCH], BF16, tag="wt_s",
                            name=f"wt{it}_{g}",
                        )
                        nc.sync.dma_start(
                            wt_s[:], wt_d[:, g * WCH:(g + 1) * WCH]
                        )
                        # u_g [128, (q, j, b)] bf16; ue_g: evac staging
                        u_g = ustr.tile(
                            [128, Q * 256], BF16, tag="u", bufs=3,
                            name=f"u{it}_{g}",
                        )
                        ut = u_g.rearrange("k (q e) -> k q e", q=Q)
                        ue_g = ustr.tile(
                            [128, Q * 256], BF16, tag="ue", bufs=3,
                            name=f"ue{it}_{g}",
                        )
                        for oct_ in range(n_oct):
                            t_ps = ps_b.tile(
                                [128, TB], T_DT, tag="t_ps",
                                name=f"t{it}_{g}_{oct_}",
                            )
                            for qq in range(nq_oct):
                                q = oct_ * nq_oct + qq
                                nc.tensor.matmul(
                                    t_ps[:, qq * 256:(qq + 1) * 256],
                                    wt_s[:, q * 128:(q + 1) * 128],  # lhsT FWL
                                    vbd[:],                   # rhs [128,256]
                                    start=True, stop=True,
                                    is_transpose=TMODE,
                                )
                            # ScalarE evacuates fp32->bf16
                            nc.scalar.copy(
                                ue_g[:, oct_ * TB:(oct_ + 1) * TB], t_ps[:]
                            )
                        # one DVE mult for the whole g at 2x
                        xv = (
                            xqv[:, g, :, :]
                            .unsqueeze(2).broadcast_to([128, Q, JL, B])
                        )
                        uev = ue_g.rearrange(
                            "k (q j b) -> k q j b", q=Q, j=JL
                        )
                        uqv = u_g.rearrange(
                            "k (q j b) -> k q j b", q=Q, j=JL
                        )
                        nc.vector.tensor_mul(uqv[:], uev[:], xv[:])
                        # q-tree: 4 strided adds -> Delta = ut[:, 0, :]
                        w = Q
                        while w > 1:
                            h = w // 2
                            nc.vector.tensor_add(
                                ut[:, 0:h, :], ut[:, 0:h, :], ut[:, h:w, :],
                            )
                            w = h
                        if pending is not None:
                            post_tree(*pending)
                        pending = (g, ut)
                        # interleave v-pass work for ready quarters
                        if g >= V_LAG:
                            emit_v(g - V_LAG)
                    post_tree(*pending)
                    for ich in range(ICH - V_LAG, ICH):
                        emit_v(ich)
                # diag-block evac -> vT [128, 64] -> transpose -> squash
                vT = small.tile([128, B], F32, tag="vTd", name=f"vTd{it}")
                for j in range(JL):
                    nc.scalar.copy(
                        vT[j * P:(j + 1) * P, :],
                        vT4[j * P:(j + 1) * P, j * B:(j + 1) * B],
                    )
                v_ps = ps_t.tile([B, 128], F32, tag="tp", name=f"vps{it}")
                nc.tensor.transpose(v_ps[:], vT[:], identity[:])
                v_sb = small.tile([B, JL * P], F32, tag="v", name=f"v{it}")
                nc.scalar.copy(v_sb[:], v_ps[:])
                _squash(nc, small, v_sb, eps_t)
                if it == 0:
                    vbd = _build_vbd4(nc, small, ps_t, v_sb, identity)
                else:
                    nc.sync.dma_start(out_d[:], v_sb[:])

    nc.compile()
    _CACHED["nc"] = nc
    return nc


def _prep_inputs(inputs_np, W_np):
    x = np.ascontiguousarray(inputs_np)           # [B, I, Q] f32
    W = np.ascontiguousarray(W_np)                # [J, I, P, Q] f32
    xq = (
        x.reshape(B, ICH, 128, Q).transpose(2, 1, 3, 0)
        .astype(NP_BF16).reshape(128, ICH * Q * B)
    )
    in_maps = []
    for r in range(N_CORES):
        Wr = W[r * JL:(r + 1) * JL]                       # [4, I, P, Q]
        wn = (
            Wr.reshape(JL, ICH, 128, P, Q).transpose(2, 1, 4, 0, 3)
            .astype(NP_BF16).reshape(128, ICH * Q * JL * P)
        )
        wt = (
            Wr.reshape(JL, ICH, 128, P, Q)
            .transpose(0, 3, 1, 4, 2)                     # [j, p, g, q, iw]
            .astype(NP_BF16).reshape(128, I * Q)
        )
        in_maps.append(
            {
                "wn": np.ascontiguousarray(wn),
                "wt": np.ascontiguousarray(wt),
                "xq": np.ascontiguousarray(xq),
            }
        )
    return in_maps


def kernel(inputs, W, _trace=False):
    nc = build_kernel()
    in_maps = _prep_inputs(np.asarray(inputs), np.asarray(W))
    res = run_bass_kernel_spmd(nc, in_maps, list(range(N_CORES)), trace=_trace)
    out = np.concatenate(
        [res.results[r]["o"].reshape(B, JL, P) for r in range(N_CORES)], axis=1
    )
    if _trace:
        kernel.last_exec_ns = res.exec_time_ns
        kernel.last_results = res
    return out.astype(np.float32)
